# revision 18
# baseline (speedup 1.0000x reference)
"""GPTBigCode MQA causal attention block on 8 TRN2 NeuronCores.

Tensor-parallel over heads: each core computes 4 of 32 query heads (the single
KV head is replicated), row-parallel c_proj, partial outputs summed on host.

Layout strategy: the QKV projection runs TRANSPOSED (stationary w1 tiles,
moving x^T supertiles) so Q^T/K^T arrive directly in [dh, t] layout for
attention -- no PE transposes, no DRAM round-trip; only V needs transposing
(32 small PE transposes). All matmuls run in bf16 (1 col/cycle at any width,
so the narrow diagonal causal blocks pay no fp32r penalty). Biases are fused
into the PSUM-eviction activations. Attention scores are computed transposed
([k_part, q_free]); softmax denominators come from a ones-vector matmul and
P@V needs no transposes; softmax skips max-subtraction (unit-variance logits
cannot overflow fp32 exp). Scores are emitted two tiles ahead of P@V so the
scalar-engine exp never stalls the PE, and each tile's c_proj is deferred by
one attention head to paper over the softmax-normalization latency. Inputs
stream as one DMA per 512-token supertile and outputs as one DMA per 8
model-dim tiles, keeping the sync engine (DMA descriptor generation) far off
the critical path so the PE stays continuously busy and its clock stays at
the ramped p-state.
"""

import numpy as np
from contextlib import ExitStack

import ml_dtypes
import concourse.bass as bass
import concourse.tile as tile
from concourse import mybir
from concourse.bass_utils import run_bass_kernel_spmd
from concourse.masks import make_identity

B, S, D = 2, 2048, 4096
H, DH = 32, 128
KV_DIM = DH
NCORES = 8
HC = H // NCORES          # 4 heads per core
DQC = HC * DH             # 512 q-dims per core
T = B * S                 # 4096 tokens
E1 = DQC + 2 * KV_DIM     # 768 = per-core QKV output dims
NE = E1 // 128            # 6 e-tiles (4 q heads, k, v)
P = 128
NKD = D // P              # 32 contraction tiles in model dim
TT = 512                  # token tile in phase A == q tile in attention
NTT = T // TT             # 8
QTILE = 512
NQJ = S // QTILE          # 4 q-tiles per batch
NKT = S // P              # 16 k tiles per batch
MEG = 8                   # me tiles per y-staging group
TSH = T // NCORES         # 512 tokens per core's KV shard
SCALE = DH ** -0.5

F32 = mybir.dt.float32
R32 = mybir.dt.float32r
BF16 = mybir.dt.bfloat16
BF_NP = ml_dtypes.bfloat16
ACTF = mybir.ActivationFunctionType
NEG = -1.0e30


def build_program():
    nc = bass.Bass(num_devices=NCORES)
    xt = nc.declare_dram_parameter("xt", [D, T], BF16, isOutput=False)
    xkv = nc.declare_dram_parameter("xkv", [D, TSH], BF16, isOutput=False)
    w1 = nc.declare_dram_parameter("w1", [D, E1], BF16, isOutput=False)
    b1 = nc.declare_dram_parameter("b1", [P, NE], F32, isOutput=False)
    b1kv = nc.declare_dram_parameter("b1kv", [1, 2 * KV_DIM], BF16,
                                     isOutput=False)
    w2 = nc.declare_dram_parameter("w2", [DQC, D], BF16, isOutput=False)
    b2 = nc.declare_dram_parameter("b2", [P, D // P], F32, isOutput=False)
    maskp = nc.declare_dram_parameter("mask", [P, P], BF16, isOutput=False)
    yt = nc.declare_dram_parameter("yt", [D, T], F32, isOutput=True)

    xt3 = xt.rearrange("(kd p) t -> p kd t", p=P)
    xkv3 = xkv.rearrange("(kd p) t -> p kd t", p=P)
    w13 = w1.rearrange("(kd p) e -> p kd e", p=P)
    w23 = w2.rearrange("(kh p) d -> p kh d", p=P)
    yt3 = yt.rearrange("(me p) t -> p me t", p=P)

    with tile.TileContext(nc) as tc:
        with ExitStack() as ctx:
            _body(ctx, tc, nc, xt3, xkv3, w13, b1, b1kv, w23, b2, maskp, yt3)
    _legalize_waits(nc)
    return nc


def _legalize_waits(nc, nop_cap=1):
    """walrus's per-instruction sync-wait budget is tiny for matmuls (LDW+MM
    lowering) and DMA pseudo-instructions. Drop redundant same-engine
    self-waits (engines execute in order), then spill excess waits onto
    same-engine NoOps inserted right before the instruction."""
    nocap = (mybir.InstNoOp,)
    f = nc.m.functions[0]
    for bb in f.blocks:
        insts = bb.instructions
        # pass 1: strip same-engine self-waits
        for i in insts:
            si = i.sync_info
            if si is None or not si.on_wait:
                continue
            ename = str(i.engine).split(".")[-1]
            if ename == "SP":
                ename = "Sync"
            kept = [w for w in si.on_wait
                    if w.sync_type != "semaphore"
                    or w.wait_reg is not None
                    or not w.ant_name.split("_")[0] == ename]
            if len(kept) != len(si.on_wait):
                si.on_wait = kept
        # pass 2: spill excess waits onto preceding nops
        idx = 0
        while idx < len(insts):
            i = insts[idx]
            si = i.sync_info
            cap = None if isinstance(i, nocap) else 1
            if cap is not None and si is not None and len(si.on_wait) > cap:
                excess = list(si.on_wait[:-cap])
                si.on_wait = list(si.on_wait[-cap:])
                while excess:
                    chunk, excess = excess[:nop_cap], excess[nop_cap:]
                    nop = mybir.InstNoOp(
                        name=nc.get_next_instruction_name(), ins=[], outs=[])
                    nop.engine = i.engine
                    nop.sync_info = mybir.SyncInfo(on_wait=chunk, on_update=[])
                    nc.register_instruction(nop)
                    insts.insert(idx, nop)
                    idx += 1
            idx += 1


def _body(ctx, tc, nc, xt3, xkv3, w13, b1, b1kv, w23, b2, maskp, yt3):
    persist = ctx.enter_context(tc.tile_pool(name="persist", bufs=1))
    qt_sb = [persist.tile([P, T], BF16, tag=f"qt{h}", name=f"qt_sb{h}")
             for h in range(HC)]
    kt_sb = persist.tile([P, T], BF16, tag="kt")          # K^T [dh, t]
    v_sb = persist.tile([P, T // P, DH], BF16, tag="v")   # V [t_part, mt, dh]
    b1_sb = persist.tile([P, NE], F32, tag="b1")
    b1kv_sb = persist.tile([1, 2 * KV_DIM], BF16, tag="b1kv")
    b2_sb = persist.tile([P, D // P], F32, tag="b2")
    mask_sb = persist.tile([P, P], BF16, tag="mask")      # additive causal mask
    ident = persist.tile([P, P], BF16, tag="ident")
    # all-ones stationary: the rowsum matmul then emits the softmax
    # denominator already broadcast across all 128 partitions for the
    # same streaming cost (output free size is what the PE pays for).
    ones_mat = persist.tile([P, P], BF16, tag="onesm")
    ones_row = persist.tile([1, P], BF16, tag="onesr")    # K=1 bias-aug lhsT

    nc.sync.dma_start(out=b1_sb[:], in_=b1[:])
    nc.sync.dma_start(out=b1kv_sb[:], in_=b1kv[:])
    nc.sync.dma_start(out=b2_sb[:], in_=b2[:])
    nc.sync.dma_start(out=mask_sb[:], in_=maskp[:])
    nc.vector.memset(ones_mat[:], 1.0)
    nc.vector.memset(ones_row[:], 1.0)
    make_identity(nc, ident[:])

    # ---------------- Phase A: QKV projection, transposed ---------------------
    # Q^T: out[e, t] = w1^T @ x^T for this core's 4 heads over ALL tokens --
    # lands directly in attention layout, no transposes, no DRAM round-trip.
    # K/V: each core projects only its 512-token shard (natural [t, e] layout,
    # full-D contraction), the shards are AllGathered across the 8 cores while
    # Q^T is still streaming, then V is consumed directly and K gets 32 small
    # PE transposes into K^T.
    with ExitStack() as actx:
        w1_pool = actx.enter_context(tc.tile_pool(name="w1", bufs=1))
        xts_pool = actx.enter_context(tc.tile_pool(name="xts", bufs=2))
        kvs_pool = actx.enter_context(tc.tile_pool(name="kvs", bufs=1))
        dram = actx.enter_context(tc.tile_pool(name="dram", bufs=1,
                                               space="DRAM"))
        ps_qkv = actx.enter_context(tc.tile_pool(name="ps_qkv", bufs=6,
                                                 space="PSUM"))
        ps_tr = actx.enter_context(tc.tile_pool(name="ps_tr", bufs=2,
                                                space="PSUM"))

        # KV shard inputs first so the PE can start on them immediately
        xkv_sb = kvs_pool.tile([P, NKD, TSH], BF16, tag="xkv")
        nc.sync.dma_start(out=xkv_sb[:], in_=xkv3[:])
        w1_sb = w1_pool.tile([P, NKD, E1], BF16, tag="w1")
        nc.sync.dma_start(out=w1_sb[:, :, DQC:], in_=w13[:, :, DQC:])
        nc.sync.dma_start(out=w1_sb[:, :, :DQC], in_=w13[:, :, :DQC])

        # per-core KV shard: kv[t, e] for t in this core's 512 tokens
        kv_stage = kvs_pool.tile([P, TSH // P, 2 * KV_DIM], BF16, tag="kvstg")
        for tch in range(TSH // P):
            ps = ps_qkv.tile([P, 2 * KV_DIM], F32, tag="qkv", name="ps_kv")
            for kd in range(NKD):
                nc.tensor.matmul(ps[:], xkv_sb[:, kd, tch * P:(tch + 1) * P],
                                 w1_sb[:, kd, DQC:],
                                 start=(kd == 0), stop=False)
            nc.tensor.matmul(ps[:], ones_row[:], b1kv_sb[:],
                             start=False, stop=True)
            nc.scalar.activation(kv_stage[:, tch, :], ps[:], ACTF.Copy)
        # shard exchange lives entirely on the gpsimd queue so the sync
        # queue (xts prefetch) and PE (Q-proj) never wait behind it
        kv_shard = dram.tile([TSH, 2 * KV_DIM], BF16, tag="kvshard")
        kv_gather = dram.tile([T, 2 * KV_DIM], BF16, tag="kvgather")
        nc.gpsimd.dma_start(
            out=kv_shard.rearrange("(tc p) e -> p tc e", p=P),
            in_=kv_stage[:])
        nc.gpsimd.collective_compute(
            "AllGather",
            mybir.AluOpType.bypass,
            replica_groups=[list(range(NCORES))],
            ins=[kv_shard.opt()],
            outs=[kv_gather.opt()],
        )
        g3 = kv_gather.rearrange("(mt p) e -> p mt e", p=P)
        nc.gpsimd.dma_start(out=v_sb[:], in_=g3[:, :, KV_DIM:])
        ktmp = kvs_pool.tile([P, T // P, P], BF16, tag="ktmp")
        nc.gpsimd.dma_start(out=ktmp[:], in_=g3[:, :, :KV_DIM])

        # Q^T projection over all tokens, this core's 4 heads. The K
        # transposes are spliced in after tt=1, by which point the AllGather
        # (~60us, overlapped with two full Q tiles) has long completed.
        for tt in range(NTT):
            xts = xts_pool.tile([P, NKD, TT], BF16, tag="xts")
            nc.sync.dma_start(out=xts[:], in_=xt3[:, :, tt * TT:(tt + 1) * TT])
            for e in range(HC):
                ps = ps_qkv.tile([P, TT], F32, tag="qkv")
                for kd in range(NKD):
                    nc.tensor.matmul(ps[:], w1_sb[:, kd, e * P:(e + 1) * P],
                                     xts[:, kd, :],
                                     start=(kd == 0), stop=(kd == NKD - 1))
                nc.scalar.activation(qt_sb[e][:, tt * TT:(tt + 1) * TT],
                                     ps[:], ACTF.Identity,
                                     bias=b1_sb[:, e:e + 1])
            if tt == 1:
                for mt in range(T // P):
                    tp = ps_tr.tile([P, P], BF16, tag="tr")
                    nc.tensor.transpose(tp[:], ktmp[:, mt, :], ident[:])
                    nc.scalar.activation(kt_sb[:, mt * P:(mt + 1) * P], tp[:],
                                         ACTF.Copy)

    # ---------------- Phase B+C: attention + c_proj ---------------------------
    # Engine split keeps every queue free-running: PE does scores/PV/rowsum/
    # bcast/c_proj; Act does only exp + y evictions; the causal-mask adds go
    # to the otherwise-idle Pool engine; the softmax-normalization chain
    # (reciprocal, bcast eviction, divide) lives entirely on DVE so it never
    # dams the Act queue in front of the next head's exps.
    w2_pool = ctx.enter_context(tc.tile_pool(name="w2", bufs=1))
    p_pool = ctx.enter_context(tc.tile_pool(name="pp", bufs=4))
    at_pool = ctx.enter_context(tc.tile_pool(name="at", bufs=8))
    ibc_pool = ctx.enter_context(tc.tile_pool(name="ibc", bufs=2))
    ystg_pool = ctx.enter_context(tc.tile_pool(name="ystg", bufs=3))
    # PSUM: shared 4-deep ring (scores / inv-bcast / c_proj) + 2 (pv accum)
    # + 2 (denominator) = 8 banks
    ps_ring = ctx.enter_context(tc.tile_pool(name="ps_ring", bufs=4,
                                             space="PSUM"))
    ps_o = ctx.enter_context(tc.tile_pool(name="ps_o", bufs=2, space="PSUM"))
    ps_den = ctx.enter_context(tc.tile_pool(name="ps_den", bufs=2,
                                            space="PSUM"))

    w2_sb = w2_pool.tile([P, HC, D], BF16, tag="w2")
    nc.sync.dma_start(out=w2_sb[:], in_=w23[:])

    def emit_attn_head(b, qj, h, at_tiles):
        tb = b * S + qj * QTILE
        nk = 4 * qj + 4
        ps_out = ps_o.tile([P, QTILE], F32, tag="o")
        den = ps_den.tile([P, QTILE], F32, tag="den")
        pt_t, qo_t = [None] * nk, [None] * nk

        def emit_scores(kk):
            r = kk - 4 * qj
            qoff = 0 if r < 0 else P * r
            c0 = b * S + kk * P
            ss = ps_ring.tile([P, QTILE], F32, tag="ps", name="ss")
            pt = p_pool.tile([P, QTILE], BF16, tag="p")
            nc.tensor.matmul(ss[:, qoff:], kt_sb[:, c0:c0 + P],
                             qt_sb[h][:, tb + qoff:tb + QTILE],
                             start=True, stop=(r < 0))
            if r >= 0:
                # causal mask folded into the PE: accumulate identity @ mask
                # onto the triangular 128x128 block (53 ns vs a DVE pass)
                nc.tensor.matmul(ss[:, qoff:qoff + P], ident[:], mask_sb[:],
                                 start=False, stop=True)
            nc.scalar.activation(pt[:, qoff:], ss[:, qoff:], ACTF.Exp,
                                 scale=SCALE)
            pt_t[kk], qo_t[kk] = pt, qoff

        def emit_pv(kk):
            qoff = qo_t[kk]
            nc.tensor.matmul(ps_out[:, qoff:], v_sb[:, b * NKT + kk, :],
                             pt_t[kk][:, qoff:],
                             start=(kk == 0), stop=(kk == nk - 1))
            nc.tensor.matmul(den[:, qoff:], ones_mat[:], pt_t[kk][:, qoff:],
                             start=(kk == 0), stop=(kk == nk - 1))

        # scores run 2 tiles ahead of pv so exp latency never stalls the PE
        emit_scores(0)
        if nk > 1:
            emit_scores(1)
        for kk in range(nk):
            if kk + 2 < nk:
                emit_scores(kk + 2)
            emit_pv(kk)

        # softmax normalization, entirely on DVE (never dams the Act queue):
        # den is already partition-broadcast, so just reciprocal + multiply.
        ibc = ibc_pool.tile([P, QTILE], F32, tag="ibc")
        nc.vector.reciprocal(ibc[:], den[:])
        at = at_pool.tile([P, QTILE], BF16, tag="at")
        nc.vector.tensor_mul(at[:], ps_out[:], ibc[:])
        at_tiles.append(at)

    def emit_cproj_half(pend, half):
        b, qj, at_tiles = pend
        tb = b * S + qj * QTILE
        for mg in range(2 * half, 2 * half + 2):
            ystg = ystg_pool.tile([P, MEG, QTILE], F32, tag="ystg")
            for i in range(MEG):
                me = mg * MEG + i
                psy = ps_ring.tile([P, QTILE], F32, tag="ps", name="psy")
                for kh in range(HC):
                    nc.tensor.matmul(psy[:], w2_sb[:, kh, me * P:(me + 1) * P],
                                     at_tiles[kh][:],
                                     start=(kh == 0), stop=(kh == HC - 1))
                nc.scalar.activation(ystg[:, i, :], psy[:], ACTF.Identity,
                                     bias=b2_sb[:, me:me + 1])
            nc.sync.dma_start(
                out=yt3[:, mg * MEG:(mg + 1) * MEG, tb:tb + QTILE],
                in_=ystg[:])

    # c_proj for each (b, qj) is deferred into the next tile, half after
    # head 0 and half after head 1, so its matmuls fill the PE while the Act
    # queue catches up on exps and its first read of at[h3] comes well after
    # the normalization chain completed.
    pending = None
    for b in range(B):
        for qj in range(NQJ):
            at_tiles = []
            for h in range(HC):
                emit_attn_head(b, qj, h, at_tiles)
                if h == 0 and pending is not None:
                    emit_cproj_half(pending, 0)
                if h == 1 and pending is not None:
                    emit_cproj_half(pending, 1)
                    pending = None
            pending = (b, qj, at_tiles)
    emit_cproj_half(pending, 0)
    emit_cproj_half(pending, 1)


_PROGRAM = None


def _get_program():
    global _PROGRAM
    if _PROGRAM is None:
        _PROGRAM = build_program()
    return _PROGRAM


def make_in_maps(hidden_states, w_qkv, b_qkv, w_proj, b_proj):
    x = np.asarray(hidden_states, dtype=np.float32).reshape(T, D)
    xt = np.ascontiguousarray(x.T.astype(BF_NP))
    # additive causal mask for the triangular block of diagonal tiles
    ki = np.arange(P)[:, None]
    qj = np.arange(P)[None, :]
    mask = np.where(ki <= qj, 0.0, NEG).astype(BF_NP)
    w_qkv = np.asarray(w_qkv, dtype=np.float32)
    b_qkv = np.asarray(b_qkv, dtype=np.float32)
    w_proj = np.asarray(w_proj, dtype=np.float32)
    b_proj = np.asarray(b_proj, dtype=np.float32)
    b2 = np.ascontiguousarray(
        (b_proj / NCORES).reshape(D // P, P).T).astype(np.float32)
    in_maps = []
    b1kv = np.ascontiguousarray(b_qkv[D:].reshape(1, 2 * KV_DIM).astype(BF_NP))
    for c in range(NCORES):
        qcols = slice(c * DQC, (c + 1) * DQC)
        w1 = np.concatenate([w_qkv[:, qcols], w_qkv[:, D:]], axis=1)
        b1cols = np.concatenate([b_qkv[qcols], b_qkv[D:]])
        b1 = np.ascontiguousarray(b1cols.reshape(NE, P).T).astype(np.float32)
        w2 = w_proj[c * DQC:(c + 1) * DQC, :]
        in_maps.append({
            "xt": xt,
            "xkv": np.ascontiguousarray(xt[:, c * TSH:(c + 1) * TSH]),
            "w1": np.ascontiguousarray(w1.astype(BF_NP)),
            "b1": b1,
            "b1kv": b1kv,
            "w2": np.ascontiguousarray(w2.astype(BF_NP)),
            "b2": b2,
            "mask": mask,
        })
    return in_maps


def kernel(hidden_states, w_qkv, b_qkv, w_proj, b_proj):
    nc = _get_program()
    in_maps = make_in_maps(hidden_states, w_qkv, b_qkv, w_proj, b_proj)
    res = run_bass_kernel_spmd(nc, in_maps, list(range(NCORES)))
    yts = [np.asarray(r["yt"], dtype=np.float32) for r in res.results]
    y = np.add.reduce(yts).T
    return np.ascontiguousarray(y.reshape(B, S, D))


# revision 22
# speedup vs baseline: 1.0164x; 1.0164x over previous
"""GPTBigCode MQA causal attention block on 8 TRN2 NeuronCores.

Tensor-parallel over heads: each core computes 4 of 32 query heads (the single
KV head is replicated), row-parallel c_proj, partial outputs summed on host.

Layout strategy: the QKV projection runs TRANSPOSED (stationary w1 tiles,
moving x^T supertiles) so Q^T/K^T arrive directly in [dh, t] layout for
attention -- no PE transposes, no DRAM round-trip; only V needs transposing
(32 small PE transposes). All matmuls run in bf16 (1 col/cycle at any width,
so the narrow diagonal causal blocks pay no fp32r penalty). Biases are fused
into the PSUM-eviction activations. Attention scores are computed transposed
([k_part, q_free]); softmax denominators come from a ones-vector matmul and
P@V needs no transposes; softmax skips max-subtraction (unit-variance logits
cannot overflow fp32 exp). Scores are emitted two tiles ahead of P@V so the
scalar-engine exp never stalls the PE, and each tile's c_proj is deferred by
one attention head to paper over the softmax-normalization latency. Inputs
stream as one DMA per 512-token supertile and outputs as one DMA per 8
model-dim tiles, keeping the sync engine (DMA descriptor generation) far off
the critical path so the PE stays continuously busy and its clock stays at
the ramped p-state.
"""

import numpy as np
from contextlib import ExitStack

import ml_dtypes
import concourse.bass as bass
import concourse.tile as tile
from concourse import mybir
from concourse.bass_utils import run_bass_kernel_spmd
from concourse.masks import make_identity

B, S, D = 2, 2048, 4096
H, DH = 32, 128
KV_DIM = DH
NCORES = 8
HC = H // NCORES          # 4 heads per core
DQC = HC * DH             # 512 q-dims per core
T = B * S                 # 4096 tokens
E1 = DQC + 2 * KV_DIM     # 768 = per-core QKV output dims
NE = E1 // 128            # 6 e-tiles (4 q heads, k, v)
P = 128
NKD = D // P              # 32 contraction tiles in model dim
TT = 512                  # token tile in phase A == q tile in attention
NTT = T // TT             # 8
QTILE = 512
NQJ = S // QTILE          # 4 q-tiles per batch
NKT = S // P              # 16 k tiles per batch
MEG = 8                   # me tiles per y-staging group
TSH = T // NCORES         # 512 tokens per core's KV shard
SCALE = DH ** -0.5

F32 = mybir.dt.float32
R32 = mybir.dt.float32r
BF16 = mybir.dt.bfloat16
BF_NP = ml_dtypes.bfloat16
ACTF = mybir.ActivationFunctionType
NEG = -1.0e30


def build_program():
    nc = bass.Bass(num_devices=NCORES)
    xt = nc.declare_dram_parameter("xt", [D, T], BF16, isOutput=False)
    xkv = nc.declare_dram_parameter("xkv", [D, TSH], BF16, isOutput=False)
    w1 = nc.declare_dram_parameter("w1", [D, E1], BF16, isOutput=False)
    b1 = nc.declare_dram_parameter("b1", [P, NE], F32, isOutput=False)
    b1kv = nc.declare_dram_parameter("b1kv", [1, 2 * KV_DIM], BF16,
                                     isOutput=False)
    w2 = nc.declare_dram_parameter("w2", [DQC, D], BF16, isOutput=False)
    b2 = nc.declare_dram_parameter("b2", [P, D // P], F32, isOutput=False)
    maskp = nc.declare_dram_parameter("mask", [P, P], BF16, isOutput=False)
    yt = nc.declare_dram_parameter("yt", [D, T], F32, isOutput=True)

    xt3 = xt.rearrange("(kd p) t -> p kd t", p=P)
    xkv3 = xkv.rearrange("(kd p) t -> p kd t", p=P)
    w13 = w1.rearrange("(kd p) e -> p kd e", p=P)
    w23 = w2.rearrange("(kh p) d -> p kh d", p=P)
    yt3 = yt.rearrange("(me p) t -> p me t", p=P)

    with tile.TileContext(nc) as tc:
        with ExitStack() as ctx:
            _body(ctx, tc, nc, xt3, xkv3, w13, b1, b1kv, w23, b2, maskp, yt3)
    _legalize_waits(nc)
    return nc


def _legalize_waits(nc, nop_cap=1):
    """walrus's per-instruction sync-wait budget is tiny for matmuls (LDW+MM
    lowering) and DMA pseudo-instructions. Drop redundant same-engine
    self-waits (engines execute in order), then spill excess waits onto
    same-engine NoOps inserted right before the instruction."""
    nocap = (mybir.InstNoOp,)
    f = nc.m.functions[0]
    for bb in f.blocks:
        insts = bb.instructions
        # pass 1: strip same-engine self-waits
        for i in insts:
            si = i.sync_info
            if si is None or not si.on_wait:
                continue
            ename = str(i.engine).split(".")[-1]
            if ename == "SP":
                ename = "Sync"
            kept = [w for w in si.on_wait
                    if w.sync_type != "semaphore"
                    or w.wait_reg is not None
                    or not w.ant_name.split("_")[0] == ename]
            if len(kept) != len(si.on_wait):
                si.on_wait = kept
        # pass 2: spill excess waits onto preceding nops
        idx = 0
        while idx < len(insts):
            i = insts[idx]
            si = i.sync_info
            cap = None if isinstance(i, nocap) else 1
            if cap is not None and si is not None and len(si.on_wait) > cap:
                excess = list(si.on_wait[:-cap])
                si.on_wait = list(si.on_wait[-cap:])
                while excess:
                    chunk, excess = excess[:nop_cap], excess[nop_cap:]
                    nop = mybir.InstNoOp(
                        name=nc.get_next_instruction_name(), ins=[], outs=[])
                    nop.engine = i.engine
                    nop.sync_info = mybir.SyncInfo(on_wait=chunk, on_update=[])
                    nc.register_instruction(nop)
                    insts.insert(idx, nop)
                    idx += 1
            idx += 1


def _body(ctx, tc, nc, xt3, xkv3, w13, b1, b1kv, w23, b2, maskp, yt3):
    persist = ctx.enter_context(tc.tile_pool(name="persist", bufs=1))
    qt_sb = [persist.tile([P, T], BF16, tag=f"qt{h}", name=f"qt_sb{h}")
             for h in range(HC)]
    kt_sb = persist.tile([P, T], BF16, tag="kt")          # K^T [dh, t]
    v_sb = persist.tile([P, T // P, DH], BF16, tag="v")   # V [t_part, mt, dh]
    b1_sb = persist.tile([P, NE], F32, tag="b1")
    b1kv_sb = persist.tile([1, 2 * KV_DIM], BF16, tag="b1kv")
    b2_sb = persist.tile([P, D // P], F32, tag="b2")
    mask_sb = persist.tile([P, P], BF16, tag="mask")      # additive causal mask
    ident = persist.tile([P, P], BF16, tag="ident")
    # all-ones stationary: the rowsum matmul then emits the softmax
    # denominator already broadcast across all 128 partitions for the
    # same streaming cost (output free size is what the PE pays for).
    ones_mat = persist.tile([P, P], BF16, tag="onesm")
    ones_row = persist.tile([1, P], BF16, tag="onesr")    # K=1 bias-aug lhsT

    # constants ride the gpsimd DMA queue (Q0) so the sync queue (Q1) is
    # free to start streaming x^T tiles at t=0
    nc.gpsimd.dma_start(out=b1_sb[:], in_=b1[:])
    nc.gpsimd.dma_start(out=b1kv_sb[:], in_=b1kv[:])
    nc.gpsimd.dma_start(out=b2_sb[:], in_=b2[:])
    nc.gpsimd.dma_start(out=mask_sb[:], in_=maskp[:])
    nc.vector.memset(ones_mat[:], 1.0)
    nc.vector.memset(ones_row[:], 1.0)
    make_identity(nc, ident[:])

    # ---------------- Phase A: QKV projection, transposed ---------------------
    # Q^T: out[e, t] = w1^T @ x^T for this core's 4 heads over ALL tokens --
    # lands directly in attention layout, no transposes, no DRAM round-trip.
    # K/V: each core projects only its 512-token shard (natural [t, e] layout,
    # full-D contraction), the shards are AllGathered across the 8 cores while
    # Q^T is still streaming, then V is consumed directly and K gets 32 small
    # PE transposes into K^T.
    with ExitStack() as actx:
        w1_pool = actx.enter_context(tc.tile_pool(name="w1", bufs=1))
        xts_pool = actx.enter_context(tc.tile_pool(name="xts", bufs=2))
        kvs_pool = actx.enter_context(tc.tile_pool(name="kvs", bufs=1))
        dram = actx.enter_context(tc.tile_pool(name="dram", bufs=1,
                                               space="DRAM"))
        ps_qkv = actx.enter_context(tc.tile_pool(name="ps_qkv", bufs=6,
                                                 space="PSUM"))
        ps_tr = actx.enter_context(tc.tile_pool(name="ps_tr", bufs=2,
                                                space="PSUM"))

        # startup loads spread across three DMA queues so they overlap:
        # xkv on gpsimd (Q0), w1 on scalar (Q10), xts stream on sync (Q1)
        xkv_sb = kvs_pool.tile([P, NKD, TSH], BF16, tag="xkv")
        nc.gpsimd.dma_start(out=xkv_sb[:], in_=xkv3[:])
        w1_sb = w1_pool.tile([P, NKD, E1], BF16, tag="w1")
        nc.scalar.dma_start(out=w1_sb[:, :, DQC:], in_=w13[:, :, DQC:])
        nc.scalar.dma_start(out=w1_sb[:, :, :DQC], in_=w13[:, :, :DQC])

        # per-core KV shard: kv[t, e] for t in this core's 512 tokens
        kv_stage = kvs_pool.tile([P, TSH // P, 2 * KV_DIM], BF16, tag="kvstg")
        for tch in range(TSH // P):
            ps = ps_qkv.tile([P, 2 * KV_DIM], F32, tag="qkv", name="ps_kv")
            for kd in range(NKD):
                nc.tensor.matmul(ps[:], xkv_sb[:, kd, tch * P:(tch + 1) * P],
                                 w1_sb[:, kd, DQC:],
                                 start=(kd == 0), stop=False)
            nc.tensor.matmul(ps[:], ones_row[:], b1kv_sb[:],
                             start=False, stop=True)
            nc.scalar.activation(kv_stage[:, tch, :], ps[:], ACTF.Copy)
        # shard exchange lives entirely on the gpsimd queue so the sync
        # queue (xts prefetch) and PE (Q-proj) never wait behind it
        kv_shard = dram.tile([TSH, 2 * KV_DIM], BF16, tag="kvshard")
        kv_gather = dram.tile([T, 2 * KV_DIM], BF16, tag="kvgather")
        nc.gpsimd.dma_start(
            out=kv_shard.rearrange("(tc p) e -> p tc e", p=P),
            in_=kv_stage[:])
        nc.gpsimd.collective_compute(
            "AllGather",
            mybir.AluOpType.bypass,
            replica_groups=[list(range(NCORES))],
            ins=[kv_shard.opt()],
            outs=[kv_gather.opt()],
        )
        g3 = kv_gather.rearrange("(mt p) e -> p mt e", p=P)
        nc.gpsimd.dma_start(out=v_sb[:], in_=g3[:, :, KV_DIM:])
        ktmp = kvs_pool.tile([P, T // P, P], BF16, tag="ktmp")
        nc.gpsimd.dma_start(out=ktmp[:], in_=g3[:, :, :KV_DIM])

        # Q^T projection over all tokens, this core's 4 heads. The K
        # transposes are spliced in after tt=3, by which point the AllGather
        # (~60us, overlapped with three full Q tiles) has long completed.
        for tt in range(NTT):
            xts = xts_pool.tile([P, NKD, TT], BF16, tag="xts")
            nc.sync.dma_start(out=xts[:], in_=xt3[:, :, tt * TT:(tt + 1) * TT])
            for e in range(HC):
                ps = ps_qkv.tile([P, TT], F32, tag="qkv")
                for kd in range(NKD):
                    nc.tensor.matmul(ps[:], w1_sb[:, kd, e * P:(e + 1) * P],
                                     xts[:, kd, :],
                                     start=(kd == 0), stop=(kd == NKD - 1))
                nc.scalar.activation(qt_sb[e][:, tt * TT:(tt + 1) * TT],
                                     ps[:], ACTF.Identity,
                                     bias=b1_sb[:, e:e + 1])
            if tt == 3:
                for mt in range(T // P):
                    tp = ps_tr.tile([P, P], BF16, tag="tr")
                    nc.tensor.transpose(tp[:], ktmp[:, mt, :], ident[:])
                    nc.scalar.activation(kt_sb[:, mt * P:(mt + 1) * P], tp[:],
                                         ACTF.Copy)

    # ---------------- Phase B+C: attention + c_proj ---------------------------
    # Engine split keeps every queue free-running: PE does scores/PV/rowsum/
    # bcast/c_proj; Act does only exp + y evictions; the causal-mask adds go
    # to the otherwise-idle Pool engine; the softmax-normalization chain
    # (reciprocal, bcast eviction, divide) lives entirely on DVE so it never
    # dams the Act queue in front of the next head's exps.
    w2_pool = ctx.enter_context(tc.tile_pool(name="w2", bufs=1))
    p_pool = ctx.enter_context(tc.tile_pool(name="pp", bufs=4))
    at_pool = ctx.enter_context(tc.tile_pool(name="at", bufs=8))
    ibc_pool = ctx.enter_context(tc.tile_pool(name="ibc", bufs=2))
    ystg_pool = ctx.enter_context(tc.tile_pool(name="ystg", bufs=3))
    # PSUM: shared 4-deep ring (scores / inv-bcast / c_proj) + 2 (pv accum)
    # + 2 (denominator) = 8 banks
    ps_ring = ctx.enter_context(tc.tile_pool(name="ps_ring", bufs=4,
                                             space="PSUM"))
    ps_o = ctx.enter_context(tc.tile_pool(name="ps_o", bufs=2, space="PSUM"))
    ps_den = ctx.enter_context(tc.tile_pool(name="ps_den", bufs=2,
                                            space="PSUM"))

    w2_sb = w2_pool.tile([P, HC, D], BF16, tag="w2")
    nc.sync.dma_start(out=w2_sb[:], in_=w23[:])

    def emit_attn_head(b, qj, h, at_tiles):
        tb = b * S + qj * QTILE
        nk = 4 * qj + 4
        ps_out = ps_o.tile([P, QTILE], F32, tag="o")
        den = ps_den.tile([P, QTILE], F32, tag="den")
        pt_t, qo_t = [None] * nk, [None] * nk

        def emit_scores(kk):
            r = kk - 4 * qj
            qoff = 0 if r < 0 else P * r
            c0 = b * S + kk * P
            ss = ps_ring.tile([P, QTILE], F32, tag="ps", name="ss")
            pt = p_pool.tile([P, QTILE], BF16, tag="p")
            nc.tensor.matmul(ss[:, qoff:], kt_sb[:, c0:c0 + P],
                             qt_sb[h][:, tb + qoff:tb + QTILE],
                             start=True, stop=(r < 0))
            if r >= 0:
                # causal mask folded into the PE: accumulate identity @ mask
                # onto the triangular 128x128 block (53 ns vs a DVE pass)
                nc.tensor.matmul(ss[:, qoff:qoff + P], ident[:], mask_sb[:],
                                 start=False, stop=True)
            nc.scalar.activation(pt[:, qoff:], ss[:, qoff:], ACTF.Exp,
                                 scale=SCALE)
            pt_t[kk], qo_t[kk] = pt, qoff

        def emit_pv(kk):
            qoff = qo_t[kk]
            nc.tensor.matmul(ps_out[:, qoff:], v_sb[:, b * NKT + kk, :],
                             pt_t[kk][:, qoff:],
                             start=(kk == 0), stop=(kk == nk - 1))
            nc.tensor.matmul(den[:, qoff:], ones_mat[:], pt_t[kk][:, qoff:],
                             start=(kk == 0), stop=(kk == nk - 1))

        # scores run 2 tiles ahead of pv so exp latency never stalls the PE
        emit_scores(0)
        if nk > 1:
            emit_scores(1)
        for kk in range(nk):
            if kk + 2 < nk:
                emit_scores(kk + 2)
            emit_pv(kk)

        # softmax normalization, entirely on DVE (never dams the Act queue):
        # den is already partition-broadcast, so just reciprocal + multiply.
        ibc = ibc_pool.tile([P, QTILE], F32, tag="ibc")
        nc.vector.reciprocal(ibc[:], den[:])
        at = at_pool.tile([P, QTILE], BF16, tag="at")
        nc.vector.tensor_mul(at[:], ps_out[:], ibc[:])
        at_tiles.append(at)

    def emit_cproj_half(pend, half):
        b, qj, at_tiles = pend
        tb = b * S + qj * QTILE
        for mg in range(2 * half, 2 * half + 2):
            ystg = ystg_pool.tile([P, MEG, QTILE], F32, tag="ystg")
            for i in range(MEG):
                me = mg * MEG + i
                psy = ps_ring.tile([P, QTILE], F32, tag="ps", name="psy")
                for kh in range(HC):
                    nc.tensor.matmul(psy[:], w2_sb[:, kh, me * P:(me + 1) * P],
                                     at_tiles[kh][:],
                                     start=(kh == 0), stop=(kh == HC - 1))
                nc.scalar.activation(ystg[:, i, :], psy[:], ACTF.Identity,
                                     bias=b2_sb[:, me:me + 1])
            nc.sync.dma_start(
                out=yt3[:, mg * MEG:(mg + 1) * MEG, tb:tb + QTILE],
                in_=ystg[:])

    # c_proj for each (b, qj) is deferred into the next tile, half after
    # head 0 and half after head 1, so its matmuls fill the PE while the Act
    # queue catches up on exps and its first read of at[h3] comes well after
    # the normalization chain completed.
    pending = None
    for b in range(B):
        for qj in range(NQJ):
            at_tiles = []
            for h in range(HC):
                emit_attn_head(b, qj, h, at_tiles)
                if h == 0 and pending is not None:
                    emit_cproj_half(pending, 0)
                if h == 1 and pending is not None:
                    emit_cproj_half(pending, 1)
                    pending = None
            pending = (b, qj, at_tiles)
    emit_cproj_half(pending, 0)
    emit_cproj_half(pending, 1)


_PROGRAM = None


def _get_program():
    global _PROGRAM
    if _PROGRAM is None:
        _PROGRAM = build_program()
    return _PROGRAM


def make_in_maps(hidden_states, w_qkv, b_qkv, w_proj, b_proj):
    x = np.asarray(hidden_states, dtype=np.float32).reshape(T, D)
    xt = np.ascontiguousarray(x.T.astype(BF_NP))
    # additive causal mask for the triangular block of diagonal tiles
    ki = np.arange(P)[:, None]
    qj = np.arange(P)[None, :]
    mask = np.where(ki <= qj, 0.0, NEG).astype(BF_NP)
    w_qkv = np.asarray(w_qkv, dtype=np.float32)
    b_qkv = np.asarray(b_qkv, dtype=np.float32)
    w_proj = np.asarray(w_proj, dtype=np.float32)
    b_proj = np.asarray(b_proj, dtype=np.float32)
    b2 = np.ascontiguousarray(
        (b_proj / NCORES).reshape(D // P, P).T).astype(np.float32)
    in_maps = []
    b1kv = np.ascontiguousarray(b_qkv[D:].reshape(1, 2 * KV_DIM).astype(BF_NP))
    for c in range(NCORES):
        qcols = slice(c * DQC, (c + 1) * DQC)
        w1 = np.concatenate([w_qkv[:, qcols], w_qkv[:, D:]], axis=1)
        b1cols = np.concatenate([b_qkv[qcols], b_qkv[D:]])
        b1 = np.ascontiguousarray(b1cols.reshape(NE, P).T).astype(np.float32)
        w2 = w_proj[c * DQC:(c + 1) * DQC, :]
        in_maps.append({
            "xt": xt,
            "xkv": np.ascontiguousarray(xt[:, c * TSH:(c + 1) * TSH]),
            "w1": np.ascontiguousarray(w1.astype(BF_NP)),
            "b1": b1,
            "b1kv": b1kv,
            "w2": np.ascontiguousarray(w2.astype(BF_NP)),
            "b2": b2,
            "mask": mask,
        })
    return in_maps


def kernel(hidden_states, w_qkv, b_qkv, w_proj, b_proj):
    nc = _get_program()
    in_maps = make_in_maps(hidden_states, w_qkv, b_qkv, w_proj, b_proj)
    res = run_bass_kernel_spmd(nc, in_maps, list(range(NCORES)))
    yts = [np.asarray(r["yt"], dtype=np.float32) for r in res.results]
    y = np.add.reduce(yts).T
    return np.ascontiguousarray(y.reshape(B, S, D))


# revision 24
# speedup vs baseline: 1.0293x; 1.0127x over previous
"""GPTBigCode MQA causal attention block on 8 TRN2 NeuronCores.

Tensor-parallel over heads: each core computes 4 of 32 query heads (the single
KV head is replicated), row-parallel c_proj, partial outputs summed on host.

Layout strategy: the QKV projection runs TRANSPOSED (stationary w1 tiles,
moving x^T supertiles) so Q^T/K^T arrive directly in [dh, t] layout for
attention -- no PE transposes, no DRAM round-trip; only V needs transposing
(32 small PE transposes). All matmuls run in bf16 (1 col/cycle at any width,
so the narrow diagonal causal blocks pay no fp32r penalty). Biases are fused
into the PSUM-eviction activations. Attention scores are computed transposed
([k_part, q_free]); softmax denominators come from a ones-vector matmul and
P@V needs no transposes; softmax skips max-subtraction (unit-variance logits
cannot overflow fp32 exp). Scores are emitted two tiles ahead of P@V so the
scalar-engine exp never stalls the PE, and each tile's c_proj is deferred by
one attention head to paper over the softmax-normalization latency. Inputs
stream as one DMA per 512-token supertile and outputs as one DMA per 8
model-dim tiles, keeping the sync engine (DMA descriptor generation) far off
the critical path so the PE stays continuously busy and its clock stays at
the ramped p-state.
"""

import numpy as np
from contextlib import ExitStack

import ml_dtypes
import concourse.bass as bass
import concourse.tile as tile
from concourse import mybir
from concourse.bass_utils import run_bass_kernel_spmd
from concourse.masks import make_identity

B, S, D = 2, 2048, 4096
H, DH = 32, 128
KV_DIM = DH
NCORES = 8
HC = H // NCORES          # 4 heads per core
DQC = HC * DH             # 512 q-dims per core
T = B * S                 # 4096 tokens
E1 = DQC + 2 * KV_DIM     # 768 = per-core QKV output dims
NE = E1 // 128            # 6 e-tiles (4 q heads, k, v)
P = 128
NKD = D // P              # 32 contraction tiles in model dim
TT = 512                  # token tile in phase A == q tile in attention
NTT = T // TT             # 8
QTILE = 512
NQJ = S // QTILE          # 4 q-tiles per batch
NKT = S // P              # 16 k tiles per batch
MEG = 8                   # me tiles per y-staging group
TSH = T // NCORES         # 512 tokens per core's KV shard
SCALE = DH ** -0.5

F32 = mybir.dt.float32
R32 = mybir.dt.float32r
BF16 = mybir.dt.bfloat16
BF_NP = ml_dtypes.bfloat16
ACTF = mybir.ActivationFunctionType
NEG = -1.0e30


def build_program():
    nc = bass.Bass(num_devices=NCORES)
    xt = nc.declare_dram_parameter("xt", [D, T], BF16, isOutput=False)
    xkv = nc.declare_dram_parameter("xkv", [D, TSH], BF16, isOutput=False)
    w1 = nc.declare_dram_parameter("w1", [D, E1], BF16, isOutput=False)
    b1 = nc.declare_dram_parameter("b1", [P, NE], F32, isOutput=False)
    b1kv = nc.declare_dram_parameter("b1kv", [1, 2 * KV_DIM], BF16,
                                     isOutput=False)
    w2 = nc.declare_dram_parameter("w2", [DQC, D], BF16, isOutput=False)
    b2 = nc.declare_dram_parameter("b2", [P, D // P], F32, isOutput=False)
    maskp = nc.declare_dram_parameter("mask", [P, P], BF16, isOutput=False)
    yt = nc.declare_dram_parameter("yt", [D, T], F32, isOutput=True)

    xt3 = xt.rearrange("(kd p) t -> p kd t", p=P)
    xkv3 = xkv.rearrange("(kd p) t -> p kd t", p=P)
    w13 = w1.rearrange("(kd p) e -> p kd e", p=P)
    w23 = w2.rearrange("(kh p) d -> p kh d", p=P)
    yt3 = yt.rearrange("(me p) t -> p me t", p=P)

    with tile.TileContext(nc) as tc:
        with ExitStack() as ctx:
            _body(ctx, tc, nc, xt3, xkv3, w13, b1, b1kv, w23, b2, maskp, yt3)
    _legalize_waits(nc)
    return nc


def _legalize_waits(nc, nop_cap=1):
    """walrus's per-instruction sync-wait budget is tiny for matmuls (LDW+MM
    lowering) and DMA pseudo-instructions. Drop redundant same-engine
    self-waits (engines execute in order), then spill excess waits onto
    same-engine NoOps inserted right before the instruction."""
    nocap = (mybir.InstNoOp,)
    f = nc.m.functions[0]
    for bb in f.blocks:
        insts = bb.instructions
        # pass 1: strip same-engine self-waits
        for i in insts:
            si = i.sync_info
            if si is None or not si.on_wait:
                continue
            ename = str(i.engine).split(".")[-1]
            if ename == "SP":
                ename = "Sync"
            kept = [w for w in si.on_wait
                    if w.sync_type != "semaphore"
                    or w.wait_reg is not None
                    or not w.ant_name.split("_")[0] == ename]
            if len(kept) != len(si.on_wait):
                si.on_wait = kept
        # pass 2: spill excess waits onto preceding nops
        idx = 0
        while idx < len(insts):
            i = insts[idx]
            si = i.sync_info
            cap = None if isinstance(i, nocap) else 1
            if cap is not None and si is not None and len(si.on_wait) > cap:
                excess = list(si.on_wait[:-cap])
                si.on_wait = list(si.on_wait[-cap:])
                while excess:
                    chunk, excess = excess[:nop_cap], excess[nop_cap:]
                    nop = mybir.InstNoOp(
                        name=nc.get_next_instruction_name(), ins=[], outs=[])
                    nop.engine = i.engine
                    nop.sync_info = mybir.SyncInfo(on_wait=chunk, on_update=[])
                    nc.register_instruction(nop)
                    insts.insert(idx, nop)
                    idx += 1
            idx += 1


def _body(ctx, tc, nc, xt3, xkv3, w13, b1, b1kv, w23, b2, maskp, yt3):
    persist = ctx.enter_context(tc.tile_pool(name="persist", bufs=1))
    qt_sb = [persist.tile([P, T], BF16, tag=f"qt{h}", name=f"qt_sb{h}")
             for h in range(HC)]
    kt_sb = persist.tile([P, T], BF16, tag="kt")          # K^T [dh, t]
    v_sb = persist.tile([P, T // P, DH], BF16, tag="v")   # V [t_part, mt, dh]
    b1_sb = persist.tile([P, NE], F32, tag="b1")
    b1kv_sb = persist.tile([1, 2 * KV_DIM], BF16, tag="b1kv")
    b2_sb = persist.tile([P, D // P], F32, tag="b2")
    mask_sb = persist.tile([P, P], BF16, tag="mask")      # additive causal mask
    ident = persist.tile([P, P], BF16, tag="ident")
    # all-ones stationary: the rowsum matmul then emits the softmax
    # denominator already broadcast across all 128 partitions for the
    # same streaming cost (output free size is what the PE pays for).
    ones_mat = persist.tile([P, P], BF16, tag="onesm")
    ones_row = persist.tile([1, P], BF16, tag="onesr")    # K=1 bias-aug lhsT

    # constants ride the gpsimd DMA queue (Q0) so the sync queue (Q1) is
    # free to start streaming x^T tiles at t=0
    nc.gpsimd.dma_start(out=b1_sb[:], in_=b1[:])
    nc.gpsimd.dma_start(out=b1kv_sb[:], in_=b1kv[:])
    nc.gpsimd.dma_start(out=b2_sb[:], in_=b2[:])
    nc.gpsimd.dma_start(out=mask_sb[:], in_=maskp[:])
    nc.vector.memset(ones_mat[:], 1.0)
    nc.vector.memset(ones_row[:], 1.0)
    make_identity(nc, ident[:])

    # ---------------- Phase A: QKV projection, transposed ---------------------
    # Q^T: out[e, t] = w1^T @ x^T for this core's 4 heads over ALL tokens --
    # lands directly in attention layout, no transposes, no DRAM round-trip.
    # K/V: each core projects only its 512-token shard (natural [t, e] layout,
    # full-D contraction), the shards are AllGathered across the 8 cores while
    # Q^T is still streaming, then V is consumed directly and K gets 32 small
    # PE transposes into K^T.
    with ExitStack() as actx:
        w1_pool = actx.enter_context(tc.tile_pool(name="w1", bufs=1))
        xts_pool = actx.enter_context(tc.tile_pool(name="xts", bufs=2))
        kvs_pool = actx.enter_context(tc.tile_pool(name="kvs", bufs=1))
        dram = actx.enter_context(tc.tile_pool(name="dram", bufs=1,
                                               space="DRAM"))
        ps_qkv = actx.enter_context(tc.tile_pool(name="ps_qkv", bufs=6,
                                                 space="PSUM"))
        ps_tr = actx.enter_context(tc.tile_pool(name="ps_tr", bufs=2,
                                                space="PSUM"))

        # startup loads spread across three DMA queues (each ~160 GB/s) so
        # they overlap: xkv chunks on gpsimd (Q0) gate the shard matmuls at
        # ~11us, w1 in per-e chunks on scalar (Q10), xts stream alternates
        # sync (Q1) / scalar so neither queue gates the Q-projection.
        xkv_sb = kvs_pool.tile([P, NKD, TSH], BF16, tag="xkv")
        for tch in range(TSH // P):
            nc.gpsimd.dma_start(out=xkv_sb[:, :, tch * P:(tch + 1) * P],
                                in_=xkv3[:, :, tch * P:(tch + 1) * P])
        w1_sb = w1_pool.tile([P, NKD, E1], BF16, tag="w1")
        nc.scalar.dma_start(out=w1_sb[:, :, DQC:], in_=w13[:, :, DQC:])
        for e in range(HC):
            nc.scalar.dma_start(out=w1_sb[:, :, e * P:(e + 1) * P],
                                in_=w13[:, :, e * P:(e + 1) * P])

        # per-core KV shard: kv[t, e] for t in this core's 512 tokens
        kv_stage = kvs_pool.tile([P, TSH // P, 2 * KV_DIM], BF16, tag="kvstg")
        for tch in range(TSH // P):
            ps = ps_qkv.tile([P, 2 * KV_DIM], F32, tag="qkv", name="ps_kv")
            for kd in range(NKD):
                nc.tensor.matmul(ps[:], xkv_sb[:, kd, tch * P:(tch + 1) * P],
                                 w1_sb[:, kd, DQC:],
                                 start=(kd == 0), stop=False)
            nc.tensor.matmul(ps[:], ones_row[:], b1kv_sb[:],
                             start=False, stop=True)
            nc.scalar.activation(kv_stage[:, tch, :], ps[:], ACTF.Copy)
        # shard exchange lives entirely on the gpsimd queue so the sync
        # queue (xts prefetch) and PE (Q-proj) never wait behind it
        kv_shard = dram.tile([TSH, 2 * KV_DIM], BF16, tag="kvshard")
        kv_gather = dram.tile([T, 2 * KV_DIM], BF16, tag="kvgather")
        nc.gpsimd.dma_start(
            out=kv_shard.rearrange("(tc p) e -> p tc e", p=P),
            in_=kv_stage[:])
        nc.gpsimd.collective_compute(
            "AllGather",
            mybir.AluOpType.bypass,
            replica_groups=[list(range(NCORES))],
            ins=[kv_shard.opt()],
            outs=[kv_gather.opt()],
        )
        g3 = kv_gather.rearrange("(mt p) e -> p mt e", p=P)
        nc.gpsimd.dma_start(out=v_sb[:], in_=g3[:, :, KV_DIM:])
        ktmp = kvs_pool.tile([P, T // P, P], BF16, tag="ktmp")
        nc.gpsimd.dma_start(out=ktmp[:], in_=g3[:, :, :KV_DIM])

        # Q^T projection over all tokens, this core's 4 heads. The K
        # transposes are spliced in after tt=3, by which point the AllGather
        # (~60us, overlapped with three full Q tiles) has long completed.
        for tt in range(NTT):
            xts = xts_pool.tile([P, NKD, TT], BF16, tag="xts")
            dma_eng = nc.sync if tt % 2 == 0 else nc.scalar
            dma_eng.dma_start(out=xts[:], in_=xt3[:, :, tt * TT:(tt + 1) * TT])
            for e in range(HC):
                ps = ps_qkv.tile([P, TT], F32, tag="qkv")
                for kd in range(NKD):
                    nc.tensor.matmul(ps[:], w1_sb[:, kd, e * P:(e + 1) * P],
                                     xts[:, kd, :],
                                     start=(kd == 0), stop=(kd == NKD - 1))
                nc.scalar.activation(qt_sb[e][:, tt * TT:(tt + 1) * TT],
                                     ps[:], ACTF.Identity,
                                     bias=b1_sb[:, e:e + 1])
            if tt == 3:
                for mt in range(T // P):
                    tp = ps_tr.tile([P, P], BF16, tag="tr")
                    nc.tensor.transpose(tp[:], ktmp[:, mt, :], ident[:])
                    nc.scalar.activation(kt_sb[:, mt * P:(mt + 1) * P], tp[:],
                                         ACTF.Copy)

    # ---------------- Phase B+C: attention + c_proj ---------------------------
    # Engine split keeps every queue free-running: PE does scores/PV/rowsum/
    # bcast/c_proj; Act does only exp + y evictions; the causal-mask adds go
    # to the otherwise-idle Pool engine; the softmax-normalization chain
    # (reciprocal, bcast eviction, divide) lives entirely on DVE so it never
    # dams the Act queue in front of the next head's exps.
    w2_pool = ctx.enter_context(tc.tile_pool(name="w2", bufs=1))
    p_pool = ctx.enter_context(tc.tile_pool(name="pp", bufs=4))
    at_pool = ctx.enter_context(tc.tile_pool(name="at", bufs=8))
    ibc_pool = ctx.enter_context(tc.tile_pool(name="ibc", bufs=2))
    ystg_pool = ctx.enter_context(tc.tile_pool(name="ystg", bufs=3))
    # PSUM: shared 4-deep ring (scores / inv-bcast / c_proj) + 2 (pv accum)
    # + 2 (denominator) = 8 banks
    ps_ring = ctx.enter_context(tc.tile_pool(name="ps_ring", bufs=4,
                                             space="PSUM"))
    ps_o = ctx.enter_context(tc.tile_pool(name="ps_o", bufs=2, space="PSUM"))
    ps_den = ctx.enter_context(tc.tile_pool(name="ps_den", bufs=2,
                                            space="PSUM"))

    w2_sb = w2_pool.tile([P, HC, D], BF16, tag="w2")
    nc.sync.dma_start(out=w2_sb[:], in_=w23[:])

    def emit_attn_head(b, qj, h, at_tiles):
        tb = b * S + qj * QTILE
        nk = 4 * qj + 4
        ps_out = ps_o.tile([P, QTILE], F32, tag="o")
        den = ps_den.tile([P, QTILE], F32, tag="den")
        pt_t, qo_t = [None] * nk, [None] * nk

        def emit_scores(kk):
            r = kk - 4 * qj
            qoff = 0 if r < 0 else P * r
            c0 = b * S + kk * P
            ss = ps_ring.tile([P, QTILE], F32, tag="ps", name="ss")
            pt = p_pool.tile([P, QTILE], BF16, tag="p")
            nc.tensor.matmul(ss[:, qoff:], kt_sb[:, c0:c0 + P],
                             qt_sb[h][:, tb + qoff:tb + QTILE],
                             start=True, stop=(r < 0))
            if r >= 0:
                # causal mask folded into the PE: accumulate identity @ mask
                # onto the triangular 128x128 block (53 ns vs a DVE pass)
                nc.tensor.matmul(ss[:, qoff:qoff + P], ident[:], mask_sb[:],
                                 start=False, stop=True)
            nc.scalar.activation(pt[:, qoff:], ss[:, qoff:], ACTF.Exp,
                                 scale=SCALE)
            pt_t[kk], qo_t[kk] = pt, qoff

        def emit_pv(kk):
            qoff = qo_t[kk]
            nc.tensor.matmul(ps_out[:, qoff:], v_sb[:, b * NKT + kk, :],
                             pt_t[kk][:, qoff:],
                             start=(kk == 0), stop=(kk == nk - 1))
            nc.tensor.matmul(den[:, qoff:], ones_mat[:], pt_t[kk][:, qoff:],
                             start=(kk == 0), stop=(kk == nk - 1))

        # scores run 2 tiles ahead of pv so exp latency never stalls the PE
        emit_scores(0)
        if nk > 1:
            emit_scores(1)
        for kk in range(nk):
            if kk + 2 < nk:
                emit_scores(kk + 2)
            emit_pv(kk)

        # softmax normalization, entirely on DVE (never dams the Act queue):
        # den is already partition-broadcast, so just reciprocal + multiply.
        ibc = ibc_pool.tile([P, QTILE], F32, tag="ibc")
        nc.vector.reciprocal(ibc[:], den[:])
        at = at_pool.tile([P, QTILE], BF16, tag="at")
        nc.vector.tensor_mul(at[:], ps_out[:], ibc[:])
        at_tiles.append(at)

    def emit_cproj_half(pend, half):
        b, qj, at_tiles = pend
        tb = b * S + qj * QTILE
        for mg in range(2 * half, 2 * half + 2):
            ystg = ystg_pool.tile([P, MEG, QTILE], F32, tag="ystg")
            for i in range(MEG):
                me = mg * MEG + i
                psy = ps_ring.tile([P, QTILE], F32, tag="ps", name="psy")
                for kh in range(HC):
                    nc.tensor.matmul(psy[:], w2_sb[:, kh, me * P:(me + 1) * P],
                                     at_tiles[kh][:],
                                     start=(kh == 0), stop=(kh == HC - 1))
                nc.scalar.activation(ystg[:, i, :], psy[:], ACTF.Identity,
                                     bias=b2_sb[:, me:me + 1])
            nc.sync.dma_start(
                out=yt3[:, mg * MEG:(mg + 1) * MEG, tb:tb + QTILE],
                in_=ystg[:])

    # c_proj for each (b, qj) is deferred into the next tile, half after
    # head 0 and half after head 1, so its matmuls fill the PE while the Act
    # queue catches up on exps and its first read of at[h3] comes well after
    # the normalization chain completed.
    pending = None
    for b in range(B):
        for qj in range(NQJ):
            at_tiles = []
            for h in range(HC):
                emit_attn_head(b, qj, h, at_tiles)
                if h == 0 and pending is not None:
                    emit_cproj_half(pending, 0)
                if h == 1 and pending is not None:
                    emit_cproj_half(pending, 1)
                    pending = None
            pending = (b, qj, at_tiles)
    emit_cproj_half(pending, 0)
    emit_cproj_half(pending, 1)


_PROGRAM = None


def _get_program():
    global _PROGRAM
    if _PROGRAM is None:
        _PROGRAM = build_program()
    return _PROGRAM


def make_in_maps(hidden_states, w_qkv, b_qkv, w_proj, b_proj):
    x = np.asarray(hidden_states, dtype=np.float32).reshape(T, D)
    xt = np.ascontiguousarray(x.T.astype(BF_NP))
    # additive causal mask for the triangular block of diagonal tiles
    ki = np.arange(P)[:, None]
    qj = np.arange(P)[None, :]
    mask = np.where(ki <= qj, 0.0, NEG).astype(BF_NP)
    w_qkv = np.asarray(w_qkv, dtype=np.float32)
    b_qkv = np.asarray(b_qkv, dtype=np.float32)
    w_proj = np.asarray(w_proj, dtype=np.float32)
    b_proj = np.asarray(b_proj, dtype=np.float32)
    b2 = np.ascontiguousarray(
        (b_proj / NCORES).reshape(D // P, P).T).astype(np.float32)
    in_maps = []
    b1kv = np.ascontiguousarray(b_qkv[D:].reshape(1, 2 * KV_DIM).astype(BF_NP))
    for c in range(NCORES):
        qcols = slice(c * DQC, (c + 1) * DQC)
        w1 = np.concatenate([w_qkv[:, qcols], w_qkv[:, D:]], axis=1)
        b1cols = np.concatenate([b_qkv[qcols], b_qkv[D:]])
        b1 = np.ascontiguousarray(b1cols.reshape(NE, P).T).astype(np.float32)
        w2 = w_proj[c * DQC:(c + 1) * DQC, :]
        in_maps.append({
            "xt": xt,
            "xkv": np.ascontiguousarray(xt[:, c * TSH:(c + 1) * TSH]),
            "w1": np.ascontiguousarray(w1.astype(BF_NP)),
            "b1": b1,
            "b1kv": b1kv,
            "w2": np.ascontiguousarray(w2.astype(BF_NP)),
            "b2": b2,
            "mask": mask,
        })
    return in_maps


def kernel(hidden_states, w_qkv, b_qkv, w_proj, b_proj):
    nc = _get_program()
    in_maps = make_in_maps(hidden_states, w_qkv, b_qkv, w_proj, b_proj)
    res = run_bass_kernel_spmd(nc, in_maps, list(range(NCORES)))
    yts = [np.asarray(r["yt"], dtype=np.float32) for r in res.results]
    y = np.add.reduce(yts).T
    return np.ascontiguousarray(y.reshape(B, S, D))
